# revision 1
# baseline (speedup 1.0000x reference)
"""Trainium2 Bass kernel: 2-layer bidirectional AllenNLP LSTM.

B=64, T=512, D_IN=512, H=500. Data-parallel over batch: 8 seqs/core x 8 cores.

Per-core design:
- Input projection per layer: x_gates = x @ W_ih'.T as a big GEMM with
  PE-transposed x tiles as the stationary operand, W streamed. Gate columns
  host-permuted to [i, f, o, 2*g] so one sigmoid activation serves everything
  (tanh(z) = 2*sigmoid(2z) - 1, g pre-scaled by 2 on the host).
- Recurrence (critical path, 2x512 sequential steps, fwd+bwd concurrent):
  gates = h.T-stationary matmul streaming W_hh'.T; x_gates(t) and bias are
  folded into the contraction via 8 identity rows + 1 ones row appended to
  K chunk 0 (chunks: 119+8+1, 127, 127, 127). fwd lands at PSUM partitions
  0-7 (tile_position col group 0), bwd at 32-39 (group 1) -> the two W
  streams run concurrently on the PE array.
- Dynamic-offset DMAs are limited (register pool), so per-16-step loop body
  only block transfers use dynamic offsets: x_gates staging loads, mask
  staging, and y block stores. Per-step x_gates injection is a static
  SBUF->SBUF DMA from staging into rows 119:127 of the streamed chunk-0 tile.
- Masking: i' = i*m, f' = f*m + (1-m); carries then need no select ops
  (bwd h_new = o*tanh(0) = 0 through the masked prefix; fwd contamination
  past the sequence end never reaches a valid output). y = h*m on GPSIMD.
- All DRAM activations are (t, b)-major; host transposes in/out.
"""

import os
import sys
from contextlib import ExitStack

import numpy as np

sys.path.insert(0, "/opt/trn_rl_repo")

import concourse.bass as bass
import concourse.bacc as bacc
import concourse.mybir as mybir
import concourse.tile as tile
from concourse.bass_utils import run_bass_kernel_spmd

B, T, D_IN, H, G = 64, 512, 512, 500, 2000  # G = 4*H
NCORES = 8
BS = B // NCORES  # 8 seqs per core
F32 = mybir.dt.float32
F32R = mybir.dt.float32r
BF16 = mybir.dt.bfloat16
ds = bass.ds
ts = bass.ts
PE = mybir.EngineType.PE
DVE = mybir.EngineType.DVE
ACT = mybir.EngineType.Activation
SIG = mybir.ActivationFunctionType.Sigmoid
MUL = mybir.AluOpType.mult
ADD = mybir.AluOpType.add
SUB = mybir.AluOpType.subtract

# K chunks of the recurrence contraction: chunk 0 = h[0:119] + 8 identity
# rows (x_gates inject) + 1 ones row (bias inject) = 128.
RCH = [(0, 119), (119, 127), (246, 127), (373, 127)]

UNROLL = 16
HB = 8  # steps per staging half-block

_env_t = os.environ.get("LSTM_T")
TT = int(_env_t) if _env_t else T  # reduced T for smoke tests


def _build_nc(t_steps: int):
    nt = t_steps // 16
    nc = bacc.Bacc("TRN2", target_bir_lowering=False, debug=False,
                   num_devices=NCORES)

    # all activations (t, b)-major
    x0 = nc.dram_tensor("x0", [t_steps, BS, D_IN], F32, kind="ExternalInput").ap()
    wih0 = nc.dram_tensor("wih0", [2, D_IN, G], F32R, kind="ExternalInput").ap()
    wih1 = nc.dram_tensor("wih1", [2, 2 * H, G], F32R, kind="ExternalInput").ap()
    whh = nc.dram_tensor("whh", [2, 2, H, G], BF16, kind="ExternalInput").ap()
    bias = nc.dram_tensor("bias", [2, 2, 1, G], BF16, kind="ExternalInput").ap()
    xgid = nc.dram_tensor("xgid", [9, 16], BF16, kind="ExternalInput").ap()
    id128 = nc.dram_tensor("id128", [128, 128], F32, kind="ExternalInput").ap()
    # mask rows 0-7: fwd mask m[b, t]; rows 8-15: time-reversed. m1 = 1 - m.
    mskc = nc.dram_tensor("mskc", [128, t_steps], F32, kind="ExternalInput").ap()
    out = nc.dram_tensor("out", [t_steps, BS, 2 * H], F32,
                         kind="ExternalOutput").ap()

    xga = nc.dram_tensor("xga", [2, t_steps, BS, G], BF16, kind="Internal").ap()
    xgb = nc.dram_tensor("xgb", [2, t_steps, BS, G], BF16, kind="Internal").ap()
    y0 = nc.dram_tensor("y0", [t_steps, BS, 2 * H], F32, kind="Internal").ap()

    with tile.TileContext(nc) as tc:
        with tc.tile_pool(name="gconst", bufs=1) as gconst:
            idt = gconst.tile([128, 128], F32, tag="idt")
            nc.sync.dma_start(idt[:, :], id128)
            _inproj(nc, tc, 0, x0, D_IN, 128, wih0, xga, idt, nt)
            tc.strict_bb_all_engine_barrier()
            _rec(nc, tc, 0, t_steps, xga, whh, bias, xgid, idt, mskc, y0)
            tc.strict_bb_all_engine_barrier()
            _inproj(nc, tc, 1, y0, 2 * H, 125, wih1, xgb, idt, nt)
            tc.strict_bb_all_engine_barrier()
            _rec(nc, tc, 1, t_steps, xgb, whh, bias, xgid, idt, mskc, out)
    nc.compile()
    return nc


def _inproj(nc, tc, layer, src, k_dim, kc, wih, xg_out, idt, nt):
    """xg_out[d, t, b, :] = src[t, b, :] @ wih[d]."""
    nk = k_dim // kc
    with ExitStack() as ctx:
        wpool = ctx.enter_context(tc.tile_pool(name=f"ipw{layer}", bufs=1))
        w_sb = []
        for d in range(2):
            row = []
            for k in range(nk):
                t = wpool.tile([kc, G], F32R, tag=f"w{d}_{k}")
                nc.sync.dma_start(t[:, :], wih[d, ts(k, kc), :])
                row.append(t)
            w_sb.append(row)
        pool = ctx.enter_context(tc.tile_pool(name=f"ip{layer}", bufs=3))
        pool2 = ctx.enter_context(tc.tile_pool(name=f"ip2_{layer}", bufs=2))
        psum = ctx.enter_context(
            tc.tile_pool(name=f"ipp{layer}", bufs=1, space="PSUM"))
        psumt = ctx.enter_context(
            tc.tile_pool(name=f"ipt{layer}", bufs=2, space="PSUM"))

        for tt in range(nt):
            xT = []
            for k in range(nk):
                xin = pool.tile([128, kc], F32, tag="xin")
                nc.sync.dma_start(xin[:, :], src[ts(tt, 16), :, ts(k, kc)])
                xtp = psumt.tile([kc, 128], F32, tag="xtp")
                nc.tensor.transpose(xtp[:, :], xin[:, :], idt[0:128, 0:128])
                xts = pool2.tile([kc, 128], F32R, tag=f"xts{k}")
                nc.vector.tensor_copy(xts[:, :], xtp[:, :])
                xT.append(xts)
            for d in range(2):
                gp = psum.tile([128, 2048], F32, tag="gp")
                for n in range(4):
                    for k in range(nk):
                        nc.tensor.matmul(
                            gp[:, 512 * n:512 * n + 500], lhsT=xT[k][:, :],
                            rhs=w_sb[d][k][:, ts(n, 500)],
                            start=(k == 0), stop=(k == nk - 1))
                    gs = pool.tile([128, 500], BF16, tag="gs")
                    nc.vector.tensor_copy(gs[:, :],
                                          gp[:, 512 * n:512 * n + 500])
                    nc.sync.dma_start(
                        xg_out[d, ts(tt, 16), :, ts(n, 500)], gs[:, :])


def _rec(nc, tc, layer, t_steps, xg, whh, bias, xgid, idt, mskc, y_out):
    """Bidirectional recurrence; y_out[t, b, 0:500] = fwd, [500:1000] = bwd."""
    with ExitStack() as ctx:
        cpool = ctx.enter_context(tc.tile_pool(name=f"rc{layer}", bufs=1))
        # Streamed W tiles. Chunk 0 even/odd (rows 119:127 get x_gates rows
        # per step via static SBUF->SBUF copy); chunks 1-3 static.
        rhs0, rhs_rest = [], []
        for d in range(2):
            pair = []
            for p in range(2):
                t = cpool.tile([128, G], BF16, tag=f"r0_{d}{p}")
                nc.sync.dma_start(t[0:119, :], whh[layer, d, 0:119, :])
                nc.sync.dma_start(t[127:128, :], bias[layer, d, :, :])
                pair.append(t)
            rhs0.append(pair)
            rest = []
            for k in range(1, 4):
                off, cnt = RCH[k]
                t = cpool.tile([cnt, G], BF16, tag=f"r{k}_{d}")
                nc.sync.dma_start(t[:, :], whh[layer, d, ds(off, cnt), :])
                rest.append(t)
            rhs_rest.append(rest)
        # Stationary h.T tiles, two alternating sets; chunk0 rows 119:128
        # hold the static identity8 + ones block.
        sets = []
        for p in range(2):
            row = []
            for k in range(4):
                t = cpool.tile([128, 40], BF16, tag=f"hT{p}_{k}")
                nc.vector.memset(t[:, :], 0.0)
                if k == 0:
                    nc.sync.dma_start(t[119:128, 0:16], xgid)
                row.append(t)
            sets.append(row)
        c_t = []
        for p in range(2):
            t = cpool.tile([64, H], F32, tag=f"c{p}")
            nc.vector.memset(t[:, :], 0.0)
            c_t.append(t)
        # staging: x_gates blocks (rows = d*64 + i*8 + b), masks, y blocks
        # (rows = d*64 + i*8 + b with bwd half t-reversed).
        stg = [cpool.tile([128, G], BF16, tag=f"stg{h}", name=f"stg{h}") for h in range(2)]
        stm = cpool.tile([128, UNROLL], F32, tag="stm")
        sty_f = [cpool.tile([64, H], F32, tag=f"styf{h}", name=f"styf{h}") for h in range(2)]
        sty_b = [cpool.tile([64, H], F32, tag=f"styb{h}", name=f"styb{h}") for h in range(2)]

        gpool = ctx.enter_context(
            tc.tile_pool(name=f"rg{layer}", bufs=1, space="PSUM"))
        tpool = ctx.enter_context(
            tc.tile_pool(name=f"rt{layer}", bufs=4, space="PSUM"))
        spool = ctx.enter_context(tc.tile_pool(name=f"rs{layer}", bufs=2))

        tc.strict_bb_all_engine_barrier()

        def body(iv0, unroll):
            assert unroll == UNROLL
            nc.gpsimd.dma_start(stm[:, :], mskc[:, ds(iv0, UNROLL)])
            for h in range(2):
                nc.sync.dma_start(stg[h][0:64, :],
                                  xg[0, ds(iv0 + h * HB, HB), :, :])
                nc.gpsimd.dma_start(
                    stg[h][64:128, :],
                    xg[1, ds(t_steps - HB - iv0 - h * HB, HB), :, :])
            for i in range(unroll):
                iv = iv0 + i
                rd, wr = i % 2, 1 - i % 2
                half, j = divmod(i, HB)
                # inject x_gates rows into the streamed chunk-0 tiles
                nc.sync.dma_start(rhs0[0][rd][119:127, :],
                                  stg[half][8 * j:8 * j + 8, :])
                jb = HB - 1 - j
                nc.sync.dma_start(rhs0[1][rd][119:127, :],
                                  stg[half][64 + 8 * jb:64 + 8 * jb + 8, :])
                gp = gpool.tile([64, 2048], F32, tag="gp")
                for d in range(2):
                    pb = 32 * d
                    cols = slice(8 * d, 8 * d + 32)
                    for n in range(4):
                        for k in (3, 1, 2, 0):
                            rt = rhs0[d][rd] if k == 0 else rhs_rest[d][k - 1]
                            kp = 128 if k == 0 else RCH[k][1]
                            nc.tensor.matmul(
                                gp[pb:pb + 32, 512 * n:512 * n + 500],
                                lhsT=sets[rd][k][0:kp, cols],
                                rhs=rt[0:kp, ts(n, 500)],
                                start=(k == 3), stop=(k == 0),
                                tile_position=(0, pb))
                gg = spool.tile([64, G], F32, tag="gg")
                gpv = gp[:, :].rearrange("p (n x) -> p n x", n=4)[:, :, 0:500]
                ggv = gg[:, :].rearrange("p (n x) -> p n x", n=4)
                nc.scalar.activation(ggv, gpv, SIG)
                m = stm[0:64, i:i + 1]
                m1 = stm[64:128, i:i + 1]
                # g' = 2*sig(2zg)-1 ; i' = i*m ; f' = f*m + (1-m)
                nc.vector.tensor_scalar(gg[:, 1500:2000], gg[:, 1500:2000],
                                        2.0, 1.0, MUL, SUB)
                nc.vector.tensor_scalar(gg[:, 0:500], gg[:, 0:500], m, None, MUL)
                nc.vector.tensor_scalar(gg[:, 500:1000], gg[:, 500:1000],
                                        m, m1, MUL, ADD)
                nc.vector.tensor_tensor(gg[:, 1500:2000], gg[:, 0:500],
                                        gg[:, 1500:2000], MUL)  # i'*g'
                nc.vector.tensor_tensor(gg[:, 500:1000], gg[:, 500:1000],
                                        c_t[rd][:, :], MUL)  # f'*c
                nc.vector.tensor_tensor(c_t[wr][:, :], gg[:, 500:1000],
                                        gg[:, 1500:2000], ADD)  # c_new
                tc_s = spool.tile([64, H], F32, tag="tc")
                nc.scalar.activation(tc_s[:, :], c_t[wr][:, :], SIG, scale=2.0)
                nc.vector.tensor_scalar(tc_s[:, :], tc_s[:, :], 2.0, 1.0,
                                        MUL, SUB)  # tanh(c)
                h_s = spool.tile([64, H], F32, tag="h")
                nc.vector.tensor_tensor(h_s[:, :], gg[:, 1000:1500],
                                        tc_s[:, :], MUL)
                # y = h*m, then stage (bwd half stored t-reversed)
                ysel = spool.tile([64, H], F32, tag="ysel")
                nc.gpsimd.tensor_scalar(ysel[:, :], h_s[:, :], m, None, MUL)
                nc.sync.dma_start(sty_f[half][8 * j:8 * j + 8, :], ysel[0:8, :])
                nc.sync.dma_start(
                    sty_b[half][8 * (HB - 1 - j):8 * (HB - j), :],
                    ysel[32:40, :])
                # transpose h into next step's stationary tiles
                for k in range(4):
                    off, cnt = RCH[k]
                    xtp = tpool.tile([128, 64], F32, tag="xtp")
                    nc.tensor.transpose(xtp[0:cnt, :], h_s[:, ds(off, cnt)],
                                        idt[0:64, 0:64])
                    nc.vector.tensor_copy(sets[wr][k][0:cnt, 0:8],
                                          xtp[0:cnt, 0:8])
                    nc.vector.tensor_copy(sets[wr][k][0:cnt, 8:16],
                                          xtp[0:cnt, 32:40])
                if j == HB - 1:  # flush this half-block of y
                    b0 = iv0 + half * HB
                    nc.scalar.dma_start(y_out[ds(b0, HB), :, 0:500],
                                        sty_f[half][:, :])
                    nc.scalar.dma_start(
                        y_out[ds(t_steps - HB - b0, HB), :, 500:1000],
                        sty_b[half][:, :])

        tc.For_i_unrolled_general(0, t_steps, 1, body, max_unroll=UNROLL,
                                  hint_engines=(PE, DVE, ACT))


def _prep_host(seqs, lengths, weights, t_steps):
    """Permute gates [i,f,g,o]->[i,f,o,2g], transpose weights, build masks."""
    def perm(w):  # [4H, K] -> rows [i, f, o, 2g], transposed -> [K, 4H]
        return np.ascontiguousarray(
            np.concatenate([w[0:500], w[500:1000], w[1500:2000],
                            2.0 * w[1000:1500]], axis=0).T)

    def pb(b):
        return np.concatenate([b[0:500], b[500:1000], b[1500:2000],
                               2.0 * b[1000:1500]])[None, :]

    bf16 = mybir.dt.np(mybir.dt.bfloat16)
    wih0 = np.stack([perm(weights["W_ih0f"]), perm(weights["W_ih0b"])])
    wih1 = np.stack([perm(weights["W_ih1f"]), perm(weights["W_ih1b"])])
    whh = np.stack([
        np.stack([perm(weights["W_hh0f"]), perm(weights["W_hh0b"])]),
        np.stack([perm(weights["W_hh1f"]), perm(weights["W_hh1b"])]),
    ]).astype(bf16)
    bias = np.stack([
        np.stack([pb(weights["b0f"]), pb(weights["b0b"])]),
        np.stack([pb(weights["b1f"]), pb(weights["b1b"])]),
    ]).astype(bf16)
    xgid = np.zeros((9, 16), bf16)
    xgid[0:8, 0:8] = np.eye(8)
    xgid[0:8, 8:16] = np.eye(8)
    xgid[8, :] = 1.0
    id128 = np.eye(128, dtype=np.float32)

    in_maps = []
    for c in range(NCORES):
        sl = slice(c * BS, (c + 1) * BS)
        m = (np.arange(t_steps)[None, :] < lengths[sl, None]).astype(np.float32)
        mskc = np.zeros((128, t_steps), np.float32)
        mskc[0:8] = m
        mskc[32:40] = m[:, ::-1]
        mskc[64:72] = 1.0 - m
        mskc[96:104] = 1.0 - m[:, ::-1]
        in_maps.append({
            "x0": np.ascontiguousarray(
                seqs[sl, :t_steps].transpose(1, 0, 2)),
            "wih0": wih0, "wih1": wih1, "whh": whh, "bias": bias,
            "xgid": xgid, "id128": id128,
            "mskc": mskc,
        })
    return in_maps


_CACHE = {}


def kernel(seqs, lengths, W_ih0f, W_hh0f, b0f, W_ih0b, W_hh0b, b0b,
           W_ih1f, W_hh1f, b1f, W_ih1b, W_hh1b, b1b, _collect=None):
    t_steps = TT
    seqs = np.asarray(seqs, np.float32)
    lengths = np.asarray(lengths)
    weights = dict(W_ih0f=W_ih0f, W_hh0f=W_hh0f, b0f=b0f, W_ih0b=W_ih0b,
                   W_hh0b=W_hh0b, b0b=b0b, W_ih1f=W_ih1f, W_hh1f=W_hh1f,
                   b1f=b1f, W_ih1b=W_ih1b, W_hh1b=W_hh1b, b1b=b1b)
    weights = {k: np.asarray(v, np.float32) for k, v in weights.items()}
    in_maps = _prep_host(seqs, lengths, weights, t_steps)

    if t_steps not in _CACHE:
        _CACHE[t_steps] = _build_nc(t_steps)
    nc = _CACHE[t_steps]

    res = run_bass_kernel_spmd(
        nc, in_maps, core_ids=list(range(NCORES)),
        trace=bool(os.environ.get("LSTM_TRACE")))
    if _collect is not None:
        _collect.append(res)
    # out is [T, BS, 2H] per core -> [B, T, 2H]
    outs = [np.asarray(r["out"]).transpose(1, 0, 2) for r in res.results]
    full = np.concatenate(outs, axis=0)
    if t_steps < T:  # smoke-test mode: pad back to full T
        pad = np.zeros((B, T, 2 * H), np.float32)
        pad[:, :t_steps] = full
        return pad
    return full


if __name__ == "__main__":
    rng = np.random.default_rng(0)
    seqs = rng.standard_normal((B, T, D_IN), dtype=np.float32)
    lengths = rng.integers(1, T + 1, (B,))
    w = {}
    d_in = D_IN
    for l in range(2):
        for d in ("f", "b"):
            w[f"W_ih{l}{d}"] = (rng.standard_normal((G, d_in)) * 0.05).astype(np.float32)
            w[f"W_hh{l}{d}"] = (rng.standard_normal((G, H)) * 0.05).astype(np.float32)
            w[f"b{l}{d}"] = np.zeros(G, np.float32)
        d_in = 2 * H
    out = kernel(seqs, lengths, **w)
    print("out", out.shape, out.dtype, float(np.abs(out).max()))



# revision 2
# speedup vs baseline: 9.1725x; 9.1725x over previous
"""Trainium2 Bass kernel v2: 2-layer bidirectional AllenNLP LSTM.

B=64, T=512, D_IN=512, H=500. Data-parallel over batch: 8 seqs/core x 8 cores.

Design (per core):
- Input projection per layer: x_gates = x @ W_ih'.T; layer-0 stationary x^T
  comes pre-transposed from the host (x0T), layer-1 stationary comes directly
  from the H-major y0 intermediate, so no on-device transposes in inproj.
  Gate columns host-permuted to [i, f, o, 2*g]. All masking is folded into
  x_gates here: i/o gates get -50*(1-m), f gets +50*(1-m), so the recurrence
  needs zero mask ops and h == y exactly (validated vs reference).
- Recurrence: per direction, gates land in one PSUM tile [128, 500] with one
  gate per 32-partition column group (i@0, f@32, o@64, 2g@96): 16 matmuls
  (4 col groups x 4 K-chunks) with h^T stationary and W_hh streamed; the 4
  col groups run concurrently on the PE array. x_gates(t)+bias are injected
  via 8 identity rows + 1 ones row in K-chunk 0 (chunks 119+9/127/127/127).
- One sigmoid activation covers all gates (tanh(z) = 2*sig(2z)-1, g-gate
  pre-scaled by 2). The activated gate tile [128, 500] is PE-transposed in 4
  chunks into an H-major PSUM tile [127, 4x128]; the whole c/h elementwise
  chain then runs on dense [127, 4, 8] APs (~32 elem/lane) and the h op
  writes the next step's h^T stationary tile directly (bf16). No h transpose,
  no mask ops, no per-step DMA except the 2 x_gates injections.
- y output (== h) is staged H-major and DMA'd to an H-major DRAM layout;
  the host un-permutes. Layer-0 y feeds layer-1 inproj as its stationary.
"""

import os
import sys
from contextlib import ExitStack

import numpy as np

sys.path.insert(0, "/opt/trn_rl_repo")

import concourse.bass as bass
import concourse.bacc as bacc
import concourse.mybir as mybir
import concourse.tile as tile
from concourse.bass_utils import run_bass_kernel_spmd

B, T, D_IN, H, G = 64, 512, 512, 500, 2000  # G = 4*H
NCORES = 8
BS = B // NCORES  # 8 seqs per core
F32 = mybir.dt.float32
F32R = mybir.dt.float32r
BF16 = mybir.dt.bfloat16
ds = bass.ds
ts = bass.ts
PE = mybir.EngineType.PE
DVE = mybir.EngineType.DVE
ACT = mybir.EngineType.Activation
SIG = mybir.ActivationFunctionType.Sigmoid
TANH = mybir.ActivationFunctionType.Tanh
MUL = mybir.AluOpType.mult
ADD = mybir.AluOpType.add
SUB = mybir.AluOpType.subtract

# K chunks of the recurrence contraction: chunk 0 = h[0:119] + 8 identity
# rows (x_gates inject) + 1 ones row (bias inject) = 128.
RCH = [(0, 119), (119, 127), (246, 127), (373, 127)]
MOFF = 50.0  # mask offset magnitude on i/f/o pre-activations

UNROLL = 16
HB = 8  # steps per staging half-block

_env_t = os.environ.get("LSTM_T")
TT = int(_env_t) if _env_t else T  # reduced T for smoke tests


def _build_nc(t_steps: int):
    nt = t_steps // 16
    nc = bacc.Bacc("TRN2", target_bir_lowering=False, debug=False,
                   num_devices=NCORES)

    x0T = nc.dram_tensor("x0T", [4, 128, t_steps * BS], F32R,
                         kind="ExternalInput").ap()
    wih0 = nc.dram_tensor("wih0", [2, D_IN, G], F32R, kind="ExternalInput").ap()
    wih1 = nc.dram_tensor("wih1", [2, 2 * H, G], BF16, kind="ExternalInput").ap()
    whh = nc.dram_tensor("whh", [2, 2, H, G], BF16, kind="ExternalInput").ap()
    bias = nc.dram_tensor("bias", [2, 2, 1, G], BF16, kind="ExternalInput").ap()
    xgid = nc.dram_tensor("xgid", [9, 16], BF16, kind="ExternalInput").ap()
    idbf = nc.dram_tensor("idbf", [128, 128], BF16, kind="ExternalInput").ap()
    moff = nc.dram_tensor("moff", [nt, 128, 2], F32, kind="ExternalInput").ap()
    out = nc.dram_tensor("out", [2, 127, 4, t_steps, BS], F32,
                         kind="ExternalOutput").ap()

    xga = nc.dram_tensor("xga", [2, t_steps, BS, G], BF16, kind="Internal").ap()
    xgb = nc.dram_tensor("xgb", [2, t_steps, BS, G], BF16, kind="Internal").ap()
    y0h = nc.dram_tensor("y0h", [2, 127, 4, t_steps, BS], BF16,
                         kind="Internal").ap()

    with tile.TileContext(nc) as tc:
        with tc.tile_pool(name="gconst", bufs=1) as gconst:
            idt = gconst.tile([128, 128], BF16, tag="idt")
            nc.sync.dma_start(idt[:, :], idbf)
            _inproj0(nc, tc, x0T, wih0, moff, xga, nt)
            tc.strict_bb_all_engine_barrier()
            _rec(nc, tc, 0, t_steps, xga, whh, bias, xgid, idt, y0h, BF16)
            tc.strict_bb_all_engine_barrier()
            _inproj1(nc, tc, y0h, wih1, moff, xgb, nt)
            tc.strict_bb_all_engine_barrier()
            _rec(nc, tc, 1, t_steps, xgb, whh, bias, xgid, idt, out, F32)
    nc.compile()
    return nc


def _emit_gates(nc, pool, gp, mofft, xg_out, d, tt):
    """PSUM gate tile [128, 2048] -> bf16 with mask offsets -> DRAM."""
    for n in range(4):
        gs = pool.tile([128, 500], BF16, tag="gs")
        sl = gp[:, 512 * n:512 * n + 500]
        if n == 3:  # 2g: no offset
            nc.vector.tensor_copy(gs[:, :], sl)
        elif n == 1:  # f: +MOFF*(1-m)
            nc.vector.tensor_scalar(gs[:, :], sl, mofft[:, 1:2], None, ADD)
        else:  # i, o: -MOFF*(1-m)
            nc.vector.tensor_scalar(gs[:, :], sl, mofft[:, 0:1], None, ADD)
        nc.sync.dma_start(xg_out[d, ts(tt, 16), :, ts(n, 500)], gs[:, :])


def _inproj0(nc, tc, x0T, wih, moff, xg_out, nt):
    """xg_out[d, t, b, :] = x[t, b, :] @ wih[d] (+ mask offsets)."""
    with ExitStack() as ctx:
        wpool = ctx.enter_context(tc.tile_pool(name="ipw0", bufs=1))
        w_sb = []
        for d in range(2):
            row = []
            for k in range(4):
                t = wpool.tile([128, G], F32R, tag=f"w{d}_{k}")
                nc.sync.dma_start(t[:, :], wih[d, ts(k, 128), :])
                row.append(t)
            w_sb.append(row)
        pool = ctx.enter_context(tc.tile_pool(name="ip0", bufs=3))
        xpool = ctx.enter_context(tc.tile_pool(name="ipx0", bufs=2))
        psum = ctx.enter_context(
            tc.tile_pool(name="ipp0", bufs=2, space="PSUM"))

        for tt in range(nt):
            mofft = pool.tile([128, 2], F32, tag="moff")
            nc.gpsimd.dma_start(mofft[:, :], moff[tt])
            xT = []
            for k in range(4):
                xt = xpool.tile([128, 128], F32R, tag=f"xT{k}")
                nc.gpsimd.dma_start(xt[:, :], x0T[k, :, ts(tt, 128)])
                xT.append(xt)
            for d in range(2):
                gp = psum.tile([128, 2048], F32, tag="gp")
                for n in range(4):
                    for k in range(4):
                        nc.tensor.matmul(
                            gp[:, 512 * n:512 * n + 500], lhsT=xT[k][:, :],
                            rhs=w_sb[d][k][:, ts(n, 500)],
                            start=(k == 0), stop=(k == 3))
                _emit_gates(nc, pool, gp, mofft, xg_out, d, tt)


def _inproj1(nc, tc, y0h, wih, moff, xg_out, nt):
    """xg_out[d, t, b, :] = y0[t, b, :] @ wih[d]; stationary from H-major y0h."""
    with ExitStack() as ctx:
        wpool = ctx.enter_context(tc.tile_pool(name="ipw1", bufs=1))
        w_sb = []
        for d in range(2):
            row = []
            for dd in range(2):
                for k in range(4):
                    off, cnt = RCH[k]
                    t = wpool.tile([cnt, G], BF16, tag=f"w{d}_{dd}{k}")
                    nc.sync.dma_start(t[:, :],
                                      wih[d, ds(500 * dd + off, cnt), :])
                    row.append(t)
            w_sb.append(row)
        pool = ctx.enter_context(tc.tile_pool(name="ip1", bufs=3))
        xpool = ctx.enter_context(tc.tile_pool(name="ipx1", bufs=2))
        psum = ctx.enter_context(
            tc.tile_pool(name="ipp1", bufs=2, space="PSUM"))

        for tt in range(nt):
            mofft = pool.tile([128, 2], F32, tag="moff")
            nc.gpsimd.dma_start(mofft[:, :], moff[tt])
            xT = []
            for dd in range(2):
                for k in range(4):
                    off, cnt = RCH[k]
                    xt = xpool.tile([cnt, 128], BF16, tag=f"xT{dd}{k}")
                    nc.gpsimd.dma_start(xt[:, :], y0h[dd, 0:cnt, k, ts(tt, 16), :])
                    xT.append(xt)
            for d in range(2):
                gp = psum.tile([128, 2048], F32, tag="gp")
                for n in range(4):
                    for c in range(8):
                        cnt = RCH[c % 4][1]
                        nc.tensor.matmul(
                            gp[:, 512 * n:512 * n + 500],
                            lhsT=xT[c][0:cnt, :],
                            rhs=w_sb[d][c][0:cnt, ts(n, 500)],
                            start=(c == 0), stop=(c == 7))
                _emit_gates(nc, pool, gp, mofft, xg_out, d, tt)


def _rec(nc, tc, layer, t_steps, xg, whh, bias, xgid, idt, y_out, y_dt):
    """Bidirectional recurrence; y == h streamed out H-major."""
    with ExitStack() as ctx:
        cpool = ctx.enter_context(tc.tile_pool(name=f"rc{layer}", bufs=1))
        # Streamed W tiles. Chunk 0 in a 4-deep rotation (rows 119:127 get
        # x_gates rows per step via static SBUF->SBUF DMA, row 127 = bias,
        # 3 steps of inject lead); chunks 1-3 static.
        rhs0, rhs_rest = [], []
        for d in range(2):
            quad = []
            for p in range(4):
                t = cpool.tile([128, G], BF16, tag=f"r0_{d}{p}")
                nc.sync.dma_start(t[0:119, :], whh[layer, d, 0:119, :])
                nc.sync.dma_start(t[127:128, :], bias[layer, d, :, :])
                quad.append(t)
            rhs0.append(quad)
            rest = []
            for k in range(1, 4):
                off, cnt = RCH[k]
                t = cpool.tile([cnt, G], BF16, tag=f"r{k}_{d}")
                nc.sync.dma_start(t[:, :], whh[layer, d, ds(off, cnt), :])
                rest.append(t)
            rhs_rest.append(rest)
        # Stationary h^T tiles [128, 64]: col 16k+8d+b; chunk-0 rows 119:128
        # hold the static identity8 + ones block (both dirs).
        sets = []
        for p in range(2):
            t = cpool.tile([128, 64], BF16, tag=f"hT{p}")
            nc.vector.memset(t[:, :], 0.0)
            nc.sync.dma_start(t[119:128, 0:16], xgid)
            sets.append(t)
        # c state + dense scratch, all [127, 32] = (4 chunks x 8 batch)
        c_t = []
        for d in range(2):
            t = cpool.tile([127, 32], F32, tag=f"c{d}")
            nc.vector.memset(t[:, :], 0.0)
            c_t.append(t)
        stg = [cpool.tile([128, G], BF16, tag=f"stg{h}", name=f"stg{h}")
               for h in range(2)]

        gpool = ctx.enter_context(
            tc.tile_pool(name=f"rg{layer}", bufs=2, space="PSUM"))
        tpool = ctx.enter_context(
            tc.tile_pool(name=f"rt{layer}", bufs=2, space="PSUM"))
        spool = ctx.enter_context(tc.tile_pool(name=f"rs{layer}", bufs=2))
        ypool = ctx.enter_context(tc.tile_pool(name=f"ry{layer}", bufs=2))

        tc.strict_bb_all_engine_barrier()

        def body(iv0, unroll):
            assert unroll == UNROLL
            for h in range(2):
                nc.gpsimd.dma_start(stg[h][0:64, :],
                                    xg[0, ds(iv0 + h * HB, HB), :, :])
                nc.gpsimd.dma_start(
                    stg[h][64:128, :],
                    xg[1, ds(t_steps - HB - iv0 - h * HB, HB), :, :])
            # y staging: [127, (k:4, i:16, b:8)] per direction, full block
            ys = [ypool.tile([127, 4 * UNROLL * BS], y_dt, tag=f"ys{d}",
                             name=f"ys{d}") for d in range(2)]
            for i in range(unroll):
                iv = iv0 + i
                rd, wr = i % 2, 1 - i % 2
                r4 = i % 4
                half, j = divmod(i, HB)
                jb = HB - 1 - j
                # inject x_gates rows into the rotating chunk-0 tiles
                # (fwd on SP queue, bwd on ACT queue)
                nc.sync.dma_start(rhs0[0][r4][119:127, :],
                                  stg[half][8 * j:8 * j + 8, :])
                nc.scalar.dma_start(rhs0[1][r4][119:127, :],
                                    stg[half][64 + 8 * jb:64 + 8 * jb + 8, :])
                # MM blocks for both directions back-to-back on the PE:
                # 4 col groups (one per gate) stream concurrently
                gp = []
                for d in range(2):
                    gpd = gpool.tile([128, 500], F32, tag=f"gp{d}",
                                     name=f"gp{d}")
                    gp.append(gpd)
                    for k in (3, 1, 2, 0):
                        rt = rhs0[d][r4] if k == 0 else rhs_rest[d][k - 1]
                        kp = 128 if k == 0 else RCH[k][1]
                        for g in range(4):
                            nc.tensor.matmul(
                                gpd[32 * g:32 * g + 8, :],
                                lhsT=sets[rd][0:kp, 16 * k + 8 * d:
                                              16 * k + 8 * d + 8],
                                rhs=rt[0:kp, ts(g, 500)],
                                start=(k == 3), stop=(k == 0),
                                tile_position=(0, 32 * g))
                # one sigmoid per direction for all gates (g pre-scaled by 2)
                gg = []
                for d in range(2):
                    ggd = spool.tile([128, 500], BF16, tag=f"gg{d}",
                                     name=f"gg{d}")
                    gg.append(ggd)
                    nc.scalar.activation(ggd[:, :], gp[d][:, :], SIG)
                # transpose gates to H-major [127, 4x128]
                xts = []
                for d in range(2):
                    xt = tpool.tile([127, 512], BF16, tag=f"xt{d}",
                                    name=f"xt{d}")
                    xts.append(xt)
                    for k in range(4):
                        off, cnt = RCH[k]
                        nc.tensor.transpose(xt[0:cnt, ts(k, 128)],
                                            gg[d][:, ds(off, cnt)],
                                            idt[0:128, 0:128])
                # dense elementwise chain, all [127, 4, 8]; the two
                # directions interleave op-by-op so sem hops hide
                xr, gsr, igr, fcr, cr, cs = [], [], [], [], [], []
                for d in range(2):
                    xr.append(xts[d][:, :].rearrange("p (k g b) -> p k g b",
                                                     k=4, g=4))
                    gsc = spool.tile([127, 32], F32, tag=f"gsc{d}",
                                     name=f"gsc{d}")
                    gsr.append(gsc[:, :].rearrange("p (k b) -> p k b", k=4))
                    igt = spool.tile([127, 32], F32, tag=f"ig{d}",
                                     name=f"ig{d}")
                    igr.append(igt[:, :].rearrange("p (k b) -> p k b", k=4))
                    fct = spool.tile([127, 32], F32, tag=f"fc{d}",
                                     name=f"fc{d}")
                    fcr.append(fct[:, :].rearrange("p (k b) -> p k b", k=4))
                    cr.append(c_t[d][:, :].rearrange("p (k b) -> p k b", k=4))
                    cs.append(spool.tile([127, 32], F32, tag=f"cs{d}",
                                         name=f"cs{d}"))
                for d in range(2):  # g' = 2*sig(2z) - 1
                    nc.vector.tensor_scalar(gsr[d], xr[d][:, :, 3, 0:8],
                                            2.0, 1.0, MUL, SUB)
                for d in range(2):  # i' * g'
                    nc.vector.tensor_tensor(igr[d], xr[d][:, :, 0, 0:8],
                                            gsr[d], MUL)
                for d in range(2):  # f' * c
                    nc.vector.tensor_tensor(fcr[d], xr[d][:, :, 1, 0:8],
                                            cr[d], MUL)
                for d in range(2):  # c_new
                    nc.vector.tensor_tensor(cr[d], igr[d], fcr[d], ADD)
                for d in range(2):  # tanh(c) on ACT
                    nc.scalar.activation(cs[d][:, :], c_t[d][:, :], TANH)
                for d in range(2):
                    csr = cs[d][:, :].rearrange("p (k b) -> p k b", k=4)
                    # h = o * tanh(c), written straight into h^T stationary
                    sr = sets[wr][:, :].rearrange("p (k e b) -> p k e b",
                                                  k=4, e=2)
                    nc.vector.tensor_tensor(sr[0:119, :, d, :],
                                            xr[d][0:119, :, 2, 0:8],
                                            csr[0:119], MUL)
                    # rows 119:127 of chunks 1-3 (32-aligned base; rows
                    # 96:119 are recomputed with identical values)
                    nc.vector.tensor_tensor(sr[96:127, 1:4, d, :],
                                            xr[d][96:127, 1:4, 2, 0:8],
                                            csr[96:127, 1:4], MUL)
                    # y == h: stage H-major (bwd t-reversed within block)
                    yr = ys[d][:, :].rearrange("p (k i b) -> p k i b",
                                               k=4, i=UNROLL)
                    nc.gpsimd.tensor_copy(
                        yr[:, :, i if d == 0 else UNROLL - 1 - i, :],
                        sr[0:127, :, d, :])
                if i == UNROLL - 1:  # flush this block of y
                    nc.sync.dma_start(y_out[0, :, :, ds(iv0, UNROLL), :],
                                      ys[0][:, :])
                    nc.sync.dma_start(
                        y_out[1, :, :, ds(t_steps - UNROLL - iv0, UNROLL), :],
                        ys[1][:, :])

        tc.For_i_unrolled_general(0, t_steps, 1, body, max_unroll=UNROLL,
                                  hint_engines=(PE, DVE, ACT))


def _prep_host(seqs, lengths, weights, t_steps):
    """Permute gates [i,f,g,o]->[i,f,o,2g], transpose weights, mask offsets."""
    def perm(w):  # [4H, K] -> rows [i, f, o, 2g], transposed -> [K, 4H]
        return np.ascontiguousarray(
            np.concatenate([w[0:500], w[500:1000], w[1500:2000],
                            2.0 * w[1000:1500]], axis=0).T)

    def pb(b):
        return np.concatenate([b[0:500], b[500:1000], b[1500:2000],
                               2.0 * b[1000:1500]])[None, :]

    bf16 = mybir.dt.np(mybir.dt.bfloat16)
    nt = t_steps // 16
    wih0 = np.stack([perm(weights["W_ih0f"]), perm(weights["W_ih0b"])])
    wih1 = np.stack([perm(weights["W_ih1f"]), perm(weights["W_ih1b"])]).astype(bf16)
    whh = np.stack([
        np.stack([perm(weights["W_hh0f"]), perm(weights["W_hh0b"])]),
        np.stack([perm(weights["W_hh1f"]), perm(weights["W_hh1b"])]),
    ]).astype(bf16)
    bias = np.stack([
        np.stack([pb(weights["b0f"]), pb(weights["b0b"])]),
        np.stack([pb(weights["b1f"]), pb(weights["b1b"])]),
    ]).astype(bf16)
    xgid = np.zeros((9, 16), bf16)
    xgid[0:8, 0:8] = np.eye(8)
    xgid[0:8, 8:16] = np.eye(8)
    xgid[8, :] = 1.0
    idbf = np.eye(128, dtype=np.float32).astype(bf16)

    in_maps = []
    for c in range(NCORES):
        sl = slice(c * BS, (c + 1) * BS)
        m = (np.arange(t_steps)[None, :] < lengths[sl, None]).astype(np.float32)
        # moff[tt, 16t*8b, {-, +}]
        offc = MOFF * (1.0 - m)  # [8, T]
        mo = np.zeros((nt, 16, BS, 2), np.float32)
        mo[:, :, :, 0] = -offc.T.reshape(nt, 16, BS)
        mo[:, :, :, 1] = offc.T.reshape(nt, 16, BS)
        x = seqs[sl, :t_steps]  # [8, T, 512]
        x0T = np.ascontiguousarray(
            x.transpose(2, 1, 0).reshape(4, 128, t_steps * BS))
        in_maps.append({
            "x0T": x0T, "wih0": wih0, "wih1": wih1, "whh": whh, "bias": bias,
            "xgid": xgid, "idbf": idbf,
            "moff": mo.reshape(nt, 128, 2),
        })
    return in_maps


_CACHE = {}


def kernel(seqs, lengths, W_ih0f, W_hh0f, b0f, W_ih0b, W_hh0b, b0b,
           W_ih1f, W_hh1f, b1f, W_ih1b, W_hh1b, b1b, _collect=None):
    t_steps = TT
    seqs = np.asarray(seqs, np.float32)
    lengths = np.asarray(lengths)
    weights = dict(W_ih0f=W_ih0f, W_hh0f=W_hh0f, b0f=b0f, W_ih0b=W_ih0b,
                   W_hh0b=W_hh0b, b0b=b0b, W_ih1f=W_ih1f, W_hh1f=W_hh1f,
                   b1f=b1f, W_ih1b=W_ih1b, W_hh1b=W_hh1b, b1b=b1b)
    weights = {k: np.asarray(v, np.float32) for k, v in weights.items()}
    in_maps = _prep_host(seqs, lengths, weights, t_steps)

    if t_steps not in _CACHE:
        _CACHE[t_steps] = _build_nc(t_steps)
    nc = _CACHE[t_steps]

    res = run_bass_kernel_spmd(
        nc, in_maps, core_ids=list(range(NCORES)),
        trace=bool(os.environ.get("LSTM_TRACE")))
    if _collect is not None:
        _collect.append(res)
    # out is [2, 127, 4, T, 8] H-major per core -> [B, T, 2H]
    full = np.zeros((B, T, 2 * H), np.float32)
    for c in range(NCORES):
        r = np.asarray(res.results[c]["out"])
        for d in range(2):
            for k in range(4):
                off, cnt = RCH[k]
                full[c * BS:(c + 1) * BS, :t_steps, 500 * d + off:
                     500 * d + off + cnt] = r[d, :cnt, k].transpose(2, 1, 0)
    return full


if __name__ == "__main__":
    rng = np.random.default_rng(0)
    seqs = rng.standard_normal((B, T, D_IN), dtype=np.float32)
    lengths = rng.integers(1, T + 1, (B,))
    w = {}
    d_in = D_IN
    for l in range(2):
        for d in ("f", "b"):
            w[f"W_ih{l}{d}"] = (rng.standard_normal((G, d_in)) * 0.05).astype(np.float32)
            w[f"W_hh{l}{d}"] = (rng.standard_normal((G, H)) * 0.05).astype(np.float32)
            w[f"b{l}{d}"] = np.zeros(G, np.float32)
        d_in = 2 * H
    out = kernel(seqs, lengths, **w)
    print("out", out.shape, out.dtype, float(np.abs(out).max()))
